# revision 12
# baseline (speedup 1.0000x reference)
"""AdaLN kernel v2 for 8 Trainium2 NeuronCores (data-parallel over tokens).

Computes, for a [B,N,768] and s [B,N,384]:
    a_n  = LayerNorm(a)                      (no affine)
    s_n  = LayerNorm(s) * ln_s_weight        (weight folded into W on host)
    gate = sigmoid(s_n @ w_gamma^T + b_gamma)
    beta = s_n @ w_beta^T
    out  = a_n * gate + beta

Design (vs f32 baseline):
  - fp16 I/O: a/s cast to fp16 on host, out stored fp16, upcast on host.
    Halves HBM traffic (31.5 MB -> 15.7 MB per core).
  - Gate projection in fp8e4 DoubleRow (2 contraction chunks per
    instruction -> half the column streams). b_gamma rides in row 0 of
    the zero-padded 4th contraction chunk: no separate bias matmuls.
  - Beta projection in fp16 (precision-critical path).
  - Stats via scale-folded accumulating passes: tensor_scalar+accum (4x)
    for means, tensor_tensor_reduce for E[s^2]/E[a^2]; single combined
    Newton rsqrt for both tensors.
  - Final "+ a_n*gate": PE streams tt into the beta psum group via an
    identity matmul, so the last vector op is a pure f32->fp16 psum
    eviction on ACT. Issue order puts it ahead of the next subtile's
    sigmoid in the ACT queue.
  - PSUM: one fused [P,1536] tile per subtile (gate 768 | beta 768,
    regions bank-aligned), plus a dedicated fp16 transpose scratch tile.

Sharding: 32768 tokens split across 8 cores (4096 each), weights
replicated. No collectives.
"""

import numpy as np
import ml_dtypes

B, N = 4, 8192
CA, CS = 768, 384
NCORES = 8
T = (B * N) // NCORES     # tokens per core = 4096
P = 128
J = 8                     # 128-token sub-tiles per macro tile
EPS = 1e-5

_CACHE = {}


def _build(t_tokens=T, debug=False):
    import concourse.bass as bass  # noqa: F401
    import concourse.tile as tile
    from concourse import bacc, mybir
    from concourse.masks import make_identity

    f32 = mybir.dt.float32
    f16 = mybir.dt.float16
    f8 = mybir.dt.float8e4
    AF = mybir.ActivationFunctionType
    OP = mybir.AluOpType
    PM = mybir.MatmulPerfMode
    NMACRO = t_tokens // (P * J)

    nc = bacc.Bacc("TRN2", target_bir_lowering=False, debug=debug)

    a_d = nc.dram_tensor("a", [t_tokens, CA], f16, kind="ExternalInput")
    s_d = nc.dram_tensor("s", [t_tokens, CS], f16, kind="ExternalInput")
    # wg8[p, pair, slot, n] = WgT[pair*256 + slot*128 + p, n]; contract row
    # 384 (pair1,slot1,p=0) holds b_gamma, rows 385..511 are zero.
    wg_d = nc.dram_tensor("wg8", [P, 2, 2, CA], f8, kind="ExternalInput")
    wb_d = nc.dram_tensor("wb16", [P, 3, CA], f16, kind="ExternalInput")
    out_d = nc.dram_tensor("out", [t_tokens, CA], f16, kind="ExternalOutput")

    a_v = a_d[:].rearrange("(m j p) c -> m p j c", j=J, p=P)
    s_v = s_d[:].rearrange("(m j p) c -> m p j c", j=J, p=P)
    o_v = out_d[:].rearrange("(m j p) c -> m p j c", j=J, p=P)

    with tile.TileContext(nc) as tc:
        with (
            tc.tile_pool(name="consts", bufs=1) as consts,
            tc.tile_pool(name="aio", bufs=2) as aio,
            tc.tile_pool(name="sio", bufs=2) as sio,
            tc.tile_pool(name="oio", bufs=3) as oio,
            tc.tile_pool(name="work", bufs=3) as work,
            tc.tile_pool(name="stats", bufs=2) as stats,
            tc.tile_pool(name="pps", bufs=4, space="PSUM") as ppsum,
        ):
            def load(m):
                a_t = aio.tile([P, J, CA], f16, tag="a_t", bufs=2)
                for h in range(0, J, 2):
                    nc.sync.dma_start(out=a_t[:, h : h + 2], in_=a_v[m, :, h : h + 2])
                s_t = sio.tile([P, J, CS], f16, tag="s_t", bufs=2)
                for h in range(0, J, 2):
                    nc.sync.dma_start(out=s_t[:, h : h + 2], in_=s_v[m, :, h : h + 2])
                return {"m": m, "a_t": a_t, "s_t": s_t}

            st_cur = load(0)
            st_next = load(1) if NMACRO > 1 else None

            ident = consts.tile([P, P], f16)
            make_identity(nc, ident)
            wg_t = consts.tile([P, 2, 2, CA], f8)
            nc.sync.dma_start(out=wg_t, in_=wg_d[:])
            wb_t = consts.tile([P, 3, CA], f16)
            nc.sync.dma_start(out=wb_t, in_=wb_d[:])
            # Static fp8 stationary tiles (double-buffered by subtile parity).
            # Chunk 3 is the pad contraction chunk: row 0 = ones (multiplies
            # the b_gamma row of wg8), rows 1..127 = 0.
            sT8 = []
            for i in range(2):
                t8 = consts.tile([P, 4, P], f8, tag=f"sT8_{i}")
                nc.vector.memset(t8[:, 3, :], 0.0)
                nc.vector.memset(t8[0:1, 3, :], 1.0)
                sT8.append(t8)
            junk = consts.tile([P, CA], f16, tag="junk")

            def newton2(vin):
                ve = stats.tile([P, 2, J, 1], f32, tag="ve", bufs=2)
                nc.vector.tensor_scalar(
                    out=ve, in0=vin, scalar1=EPS, scalar2=None, op0=OP.add
                )
                rst = stats.tile([P, 2, J, 1], f32, tag="rst", bufs=2)
                nc.vector.tensor_scalar(
                    out=rst, in0=ve, scalar1=-0.45, scalar2=1.45,
                    op0=OP.mult, op1=OP.add,
                )
                h = stats.tile([P, 2, J, 1], f32, tag="h", bufs=2)
                nc.vector.tensor_tensor(out=h, in0=rst, in1=rst, op=OP.mult)
                nc.vector.tensor_tensor(out=h, in0=h, in1=ve, op=OP.mult)
                nc.vector.tensor_scalar(
                    out=h, in0=h, scalar1=-0.5, scalar2=1.5,
                    op0=OP.mult, op1=OP.add,
                )
                nc.vector.tensor_tensor(out=rst, in0=rst, in1=h, op=OP.mult)
                return rst

            def stats_j(st, j):
                """Per-subtile stats: issued interleaved with main-loop work
                so the DVE queue never holds a monolithic stats block."""
                a_t, s_t = st["a_t"], st["s_t"]
                if "st6" not in st:
                    st6_t = stats.tile([P, J, 6], f32, tag="st6", bufs=2)
                    sta_t = stats.tile([P, J, 2, 6], f32, tag="sta", bufs=2)
                    mv_t = stats.tile([P, 2, J, 2], f32, tag="mv", bufs=2)
                    st["st6"], st["sta"], st["mv"] = st6_t, sta_t, mv_t
                st6, sta, mv = st["st6"], st["sta"], st["mv"]
                a_h = a_t.rearrange("p j (h c) -> p j h c", h=2)
                nc.vector.bn_stats(out=st6[:, j], in_=s_t[:, j])
                nc.vector.bn_stats(out=sta[:, j, 0], in_=a_h[:, j, 0])
                nc.vector.bn_stats(out=sta[:, j, 1], in_=a_h[:, j, 1])
                nc.vector.bn_aggr(out=mv[:, 0, j], in_=st6[:, j])
                nc.vector.bn_aggr(out=mv[:, 1, j], in_=sta[:, j])

            def stats_fin(st):
                mv = st["mv"]
                rst = newton2(mv[:, :, :, 1:2])
                st["mn"], st["rst"] = mv[:, :, :, 0:1], rst

            def do_stats(st):
                for j in range(J):
                    stats_j(st, j)
                stats_fin(st)

            def stage_a1(st, j):
                """s_n, transpose, evicts, gate matmuls."""
                s_t = st["s_t"]
                mn, rst = st["mn"], st["rst"]

                sn = work.tile([P, CS], f16, tag="sn", bufs=4)
                nc.vector.tensor_scalar(
                    out=sn, in0=s_t[:, j],
                    scalar1=mn[:, 0, j], scalar2=rst[:, 0, j],
                    op0=OP.subtract, op1=OP.mult,
                )
                p_t = ppsum.tile([P, 1024], f32, tag="p", bufs=4)
                trp = p_t[:, 768:960].bitcast(f16).rearrange(
                    "p (k x) -> p k x", k=3
                )
                for k in range(3):
                    nc.tensor.transpose(
                        out=trp[:, k], in_=sn[:, k * P : (k + 1) * P],
                        identity=ident,
                    )
                sT16 = work.tile([P, 3, P], f16, tag="sT16", bufs=4)
                nc.scalar.activation(out=sT16, in_=trp, func=AF.Copy)
                t8 = sT8[j % 2]
                nc.scalar.activation(out=t8[:, 0:3, :], in_=trp, func=AF.Copy)

                for c0, c1 in ((0, 512), (512, 768)):
                    nc.tensor.matmul(
                        p_t[:, c0:c1], t8[:, 0:2, :], wg_t[:, 0, :, c0:c1],
                        start=True, stop=False, perf_mode=PM.DoubleRow,
                    )
                    nc.tensor.matmul(
                        p_t[:, c0:c1], t8[:, 2:4, :], wg_t[:, 1, :, c0:c1],
                        start=False, stop=True, perf_mode=PM.DoubleRow,
                    )
                return {"p_t": p_t, "sT16": sT16, "j": j}

            def stage_a2(st, ctx):
                """sigmoid, a_n."""
                a_t = st["a_t"]
                mn, rst = st["mn"], st["rst"]
                p_t, j = ctx["p_t"], ctx["j"]
                gate = work.tile([P, CA], f16, tag="gate", bufs=4)
                nc.scalar.activation(out=gate, in_=p_t[:, 0:768], func=AF.Sigmoid)
                an = work.tile([P, CA], f16, tag="an", bufs=4)
                nc.vector.tensor_scalar(
                    out=an, in0=a_t[:, j],
                    scalar1=mn[:, 1, j], scalar2=rst[:, 1, j],
                    op0=OP.subtract, op1=OP.mult,
                )
                ctx["gate"], ctx["an"] = gate, an

            def stage_a3(ctx):
                """beta matmuls reuse the gate banks (after sigmoid), tt."""
                p_t, sT16 = ctx["p_t"], ctx["sT16"]
                for c0, c1 in ((0, 512), (512, 768)):
                    for k in range(3):
                        nc.tensor.matmul(
                            p_t[:, c0:c1], sT16[:, k], wb_t[:, k, c0:c1],
                            start=(k == 0), stop=False,
                        )
                ttv = work.tile([P, CA], f16, tag="tt", bufs=4)
                nc.vector.tensor_tensor(
                    out=ttv, in0=ctx["an"], in1=ctx["gate"], op=OP.mult
                )
                ctx["ttv"] = ttv

            def stage_b(ctx, m):
                """PE-add tt into beta psum, then ACT evict psum -> fp16."""
                p_t, ttv, j = ctx["p_t"], ctx["ttv"], ctx["j"]
                for c0, c1 in ((0, 512), (512, 768)):
                    nc.tensor.matmul(
                        p_t[:, c0:c1], ident, ttv[:, c0:c1],
                        start=False, stop=True,
                    )
                o_t = oio.tile([P, CA], f16, tag="o_t", bufs=4)
                nc.scalar.activation(
                    out=o_t, in_=p_t[:, 0:768], func=AF.Copy
                )
                nc.sync.dma_start(out=o_v[m, :, j], in_=o_t)

            do_stats(st_cur)

            # Software pipeline, ~3 subtiles in flight:
            #   a1(j) gate side | a3(j-1) beta (banks freed by sigma(j-1))
            #   | a2(j) sigmoid | b(j-2) add + evict
            p1 = None   # (st, ctx) after a1/a2, awaiting a3
            p2 = None   # (ctx, m) after a3, awaiting b
            for m in range(NMACRO):
                st = st_cur
                st_next2 = load(m + 2) if m + 2 < NMACRO else None
                for j in range(J):
                    ctx = stage_a1(st, j)
                    if p1 is not None:
                        stage_a3(p1[1])
                    stage_a2(st, ctx)
                    if p2 is not None:
                        stage_b(*p2)
                    p2 = (p1[1], p1[2]) if p1 is not None else None
                    p1 = (st, ctx, m)
                    if st_next is not None:
                        stats_j(st_next, j)
                        if j == J - 1:
                            stats_fin(st_next)
                st_cur, st_next = st_next, st_next2
            stage_a3(p1[1])
            if p2 is not None:
                stage_b(*p2)
            stage_b(p1[1], p1[2])

    nc.finalize()
    return nc


def _get_nc():
    if "nc" not in _CACHE:
        _CACHE["nc"] = _build()
    return _CACHE["nc"]


def _pack_weights(ln_s_weight, w_gamma, b_gamma, w_beta):
    f8 = ml_dtypes.float8_e4m3
    lnw = np.asarray(ln_s_weight, np.float32)
    wgT = np.ascontiguousarray(
        (np.asarray(w_gamma, np.float32) * lnw[None, :]).T
    )  # [384, 768]
    wbT = np.ascontiguousarray(
        (np.asarray(w_beta, np.float32) * lnw[None, :]).T
    )
    wg8 = np.zeros((P, 2, 2, CA), np.float32)
    for pair in range(2):
        for slot in range(2):
            c0 = pair * 256 + slot * 128
            if c0 < CS:
                wg8[:, pair, slot, :] = wgT[c0 : c0 + 128, :]
    wg8[0, 1, 1, :] = np.asarray(b_gamma, np.float32)
    wg8 = wg8.astype(f8)
    wb16 = np.ascontiguousarray(
        wbT.reshape(3, P, CA).transpose(1, 0, 2)
    ).astype(np.float16)
    return wg8, wb16


def _prep_inputs(a, s, ln_s_weight, w_gamma, b_gamma, w_beta):
    a2 = np.asarray(a, np.float32).reshape(B * N, CA).astype(np.float16)
    s2 = np.asarray(s, np.float32).reshape(B * N, CS).astype(np.float16)
    wg8, wb16 = _pack_weights(ln_s_weight, w_gamma, b_gamma, w_beta)
    in_maps = []
    for i in range(NCORES):
        in_maps.append(
            {
                "a": a2[i * T : (i + 1) * T],
                "s": s2[i * T : (i + 1) * T],
                "wg8": wg8,
                "wb16": wb16,
            }
        )
    return in_maps


def run(a, s, ln_s_weight, w_gamma, b_gamma, w_beta, trace=False, tmpdir=None):
    """Run on 8 NeuronCores; returns (output, BassKernelResults)."""
    from concourse import bass_utils

    nc = _get_nc()
    in_maps = _prep_inputs(a, s, ln_s_weight, w_gamma, b_gamma, w_beta)
    res = bass_utils.run_bass_kernel_spmd(
        nc, in_maps, core_ids=list(range(NCORES)), trace=trace, tmpdir=tmpdir
    )
    out = np.concatenate([np.asarray(r["out"]) for r in res.results], axis=0)
    return out.reshape(B, N, CA).astype(np.float32), res


def kernel(a, s, ln_s_weight, w_gamma, b_gamma, w_beta):
    out, _ = run(a, s, ln_s_weight, w_gamma, b_gamma, w_beta, trace=False)
    return out


# revision 14
# speedup vs baseline: 1.0580x; 1.0580x over previous
"""AdaLN kernel v2 for 8 Trainium2 NeuronCores (data-parallel over tokens).

Computes, for a [B,N,768] and s [B,N,384]:
    a_n  = LayerNorm(a)                      (no affine)
    s_n  = LayerNorm(s) * ln_s_weight        (weight folded into W on host)
    gate = sigmoid(s_n @ w_gamma^T + b_gamma)
    beta = s_n @ w_beta^T
    out  = a_n * gate + beta

Design (vs f32 baseline):
  - fp16 I/O: a/s cast to fp16 on host, out stored fp16, upcast on host.
    Halves HBM traffic (31.5 MB -> 15.7 MB per core).
  - Gate projection in fp8e4 DoubleRow (2 contraction chunks per
    instruction -> half the column streams). b_gamma rides in row 0 of
    the zero-padded 4th contraction chunk: no separate bias matmuls.
  - Beta projection in fp16 (precision-critical path).
  - Stats via scale-folded accumulating passes: tensor_scalar+accum (4x)
    for means, tensor_tensor_reduce for E[s^2]/E[a^2]; single combined
    Newton rsqrt for both tensors.
  - Final "+ a_n*gate": PE streams tt into the beta psum group via an
    identity matmul, so the last vector op is a pure f32->fp16 psum
    eviction on ACT. Issue order puts it ahead of the next subtile's
    sigmoid in the ACT queue.
  - PSUM: one fused [P,1536] tile per subtile (gate 768 | beta 768,
    regions bank-aligned), plus a dedicated fp16 transpose scratch tile.

Sharding: 32768 tokens split across 8 cores (4096 each), weights
replicated. No collectives.
"""

import numpy as np
import ml_dtypes

B, N = 4, 8192
CA, CS = 768, 384
NCORES = 8
T = (B * N) // NCORES     # tokens per core = 4096
P = 128
J = 8                     # 128-token sub-tiles per macro tile
EPS = 1e-5

_CACHE = {}


def _build(t_tokens=T, debug=False):
    import concourse.bass as bass  # noqa: F401
    import concourse.tile as tile
    from concourse import bacc, mybir
    from concourse.masks import make_identity

    f32 = mybir.dt.float32
    f16 = mybir.dt.float16
    f8 = mybir.dt.float8e4
    AF = mybir.ActivationFunctionType
    OP = mybir.AluOpType
    PM = mybir.MatmulPerfMode
    NMACRO = t_tokens // (P * J)

    nc = bacc.Bacc("TRN2", target_bir_lowering=False, debug=debug)

    a_d = nc.dram_tensor("a", [t_tokens, CA], f16, kind="ExternalInput")
    s_d = nc.dram_tensor("s", [t_tokens, CS], f16, kind="ExternalInput")
    # wg8[p, pair, slot, n] = WgT[pair*256 + slot*128 + p, n]; contract row
    # 384 (pair1,slot1,p=0) holds b_gamma, rows 385..511 are zero.
    wg_d = nc.dram_tensor("wg8", [P, 2, 2, CA], f8, kind="ExternalInput")
    wb_d = nc.dram_tensor("wb16", [P, 3, CA], f16, kind="ExternalInput")
    out_d = nc.dram_tensor("out", [t_tokens, CA], f16, kind="ExternalOutput")

    a_v = a_d[:].rearrange("(m j p) c -> m p j c", j=J, p=P)
    s_v = s_d[:].rearrange("(m j p) c -> m p j c", j=J, p=P)
    o_v = out_d[:].rearrange("(m j p) c -> m p j c", j=J, p=P)

    with tile.TileContext(nc) as tc:
        with (
            tc.tile_pool(name="consts", bufs=1) as consts,
            tc.tile_pool(name="aio", bufs=3) as aio,
            tc.tile_pool(name="sio", bufs=3) as sio,
            tc.tile_pool(name="oio", bufs=3) as oio,
            tc.tile_pool(name="work", bufs=3) as work,
            tc.tile_pool(name="stats", bufs=2) as stats,
            tc.tile_pool(name="pps", bufs=4, space="PSUM") as ppsum,
        ):
            def load(m):
                a_t = aio.tile([P, J, CA], f16, tag="a_t", bufs=3)
                for h in range(0, J, 2):
                    nc.sync.dma_start(out=a_t[:, h : h + 2], in_=a_v[m, :, h : h + 2])
                s_t = sio.tile([P, J, CS], f16, tag="s_t", bufs=3)
                nc.sync.dma_start(out=s_t, in_=s_v[m])
                return {"m": m, "a_t": a_t, "s_t": s_t}

            st_cur = load(0)
            st_next = load(1) if NMACRO > 1 else None

            ident = consts.tile([P, P], f16)
            make_identity(nc, ident)
            wg_t = consts.tile([P, 2, 2, CA], f8)
            nc.sync.dma_start(out=wg_t, in_=wg_d[:])
            wb_t = consts.tile([P, 3, CA], f16)
            nc.sync.dma_start(out=wb_t, in_=wb_d[:])
            # Static fp8 stationary tiles (double-buffered by subtile parity).
            # Chunk 3 is the pad contraction chunk: row 0 = ones (multiplies
            # the b_gamma row of wg8), rows 1..127 = 0.
            sT8 = []
            for i in range(2):
                t8 = consts.tile([P, 4, P], f8, tag=f"sT8_{i}")
                nc.vector.memset(t8[:, 3, :], 0.0)
                nc.vector.memset(t8[0:1, 3, :], 1.0)
                sT8.append(t8)
            junk = consts.tile([P, CA], f16, tag="junk")

            def newton2(vin):
                ve = stats.tile([P, 2, J, 1], f32, tag="ve", bufs=2)
                nc.vector.tensor_scalar(
                    out=ve, in0=vin, scalar1=EPS, scalar2=None, op0=OP.add
                )
                rst = stats.tile([P, 2, J, 1], f32, tag="rst", bufs=2)
                nc.vector.tensor_scalar(
                    out=rst, in0=ve, scalar1=-0.45, scalar2=1.45,
                    op0=OP.mult, op1=OP.add,
                )
                h = stats.tile([P, 2, J, 1], f32, tag="h", bufs=2)
                nc.vector.tensor_tensor(out=h, in0=rst, in1=rst, op=OP.mult)
                nc.vector.tensor_tensor(out=h, in0=h, in1=ve, op=OP.mult)
                nc.vector.tensor_scalar(
                    out=h, in0=h, scalar1=-0.5, scalar2=1.5,
                    op0=OP.mult, op1=OP.add,
                )
                nc.vector.tensor_tensor(out=rst, in0=rst, in1=h, op=OP.mult)
                return rst

            def stats_j(st, j):
                """Per-subtile stats: issued interleaved with main-loop work
                so the DVE queue never holds a monolithic stats block."""
                a_t, s_t = st["a_t"], st["s_t"]
                if "st6" not in st:
                    st6_t = stats.tile([P, J, 6], f32, tag="st6", bufs=2)
                    sta_t = stats.tile([P, J, 2, 6], f32, tag="sta", bufs=2)
                    mv_t = stats.tile([P, 2, J, 2], f32, tag="mv", bufs=2)
                    st["st6"], st["sta"], st["mv"] = st6_t, sta_t, mv_t
                st6, sta, mv = st["st6"], st["sta"], st["mv"]
                a_h = a_t.rearrange("p j (h c) -> p j h c", h=2)
                nc.vector.bn_stats(out=st6[:, j], in_=s_t[:, j])
                nc.vector.bn_stats(out=sta[:, j, 0], in_=a_h[:, j, 0])
                nc.vector.bn_stats(out=sta[:, j, 1], in_=a_h[:, j, 1])
                nc.vector.bn_aggr(out=mv[:, 0, j], in_=st6[:, j])
                nc.vector.bn_aggr(out=mv[:, 1, j], in_=sta[:, j])

            def stats_fin(st):
                mv = st["mv"]
                rst = newton2(mv[:, :, :, 1:2])
                st["mn"], st["rst"] = mv[:, :, :, 0:1], rst

            def do_stats(st):
                for j in range(J):
                    stats_j(st, j)
                stats_fin(st)

            def stage_a1(st, j):
                """s_n, transpose, evicts, gate matmuls."""
                s_t = st["s_t"]
                mn, rst = st["mn"], st["rst"]

                sn = work.tile([P, CS], f16, tag="sn", bufs=6)
                nc.vector.tensor_scalar(
                    out=sn, in0=s_t[:, j],
                    scalar1=mn[:, 0, j], scalar2=rst[:, 0, j],
                    op0=OP.subtract, op1=OP.mult,
                )
                p_t = ppsum.tile([P, 1024], f32, tag="p", bufs=4)
                trp = p_t[:, 768:960].bitcast(f16).rearrange(
                    "p (k x) -> p k x", k=3
                )
                for k in range(3):
                    nc.tensor.transpose(
                        out=trp[:, k], in_=sn[:, k * P : (k + 1) * P],
                        identity=ident,
                    )
                sT16 = work.tile([P, 3, P], f16, tag="sT16", bufs=6)
                nc.scalar.activation(out=sT16, in_=trp, func=AF.Copy)
                t8 = sT8[j % 2]
                nc.scalar.activation(out=t8[:, 0:3, :], in_=trp, func=AF.Copy)

                for c0, c1 in ((0, 512), (512, 768)):
                    nc.tensor.matmul(
                        p_t[:, c0:c1], t8[:, 0:2, :], wg_t[:, 0, :, c0:c1],
                        start=True, stop=False, perf_mode=PM.DoubleRow,
                    )
                    nc.tensor.matmul(
                        p_t[:, c0:c1], t8[:, 2:4, :], wg_t[:, 1, :, c0:c1],
                        start=False, stop=True, perf_mode=PM.DoubleRow,
                    )
                return {"p_t": p_t, "sT16": sT16, "j": j}

            def stage_a2(st, ctx):
                """sigmoid, a_n."""
                a_t = st["a_t"]
                mn, rst = st["mn"], st["rst"]
                p_t, j = ctx["p_t"], ctx["j"]
                gate = work.tile([P, CA], f16, tag="gate", bufs=6)
                nc.scalar.activation(out=gate, in_=p_t[:, 0:768], func=AF.Sigmoid)
                an = work.tile([P, CA], f16, tag="an", bufs=6)
                nc.vector.tensor_scalar(
                    out=an, in0=a_t[:, j],
                    scalar1=mn[:, 1, j], scalar2=rst[:, 1, j],
                    op0=OP.subtract, op1=OP.mult,
                )
                ctx["gate"], ctx["an"] = gate, an

            def stage_a3(ctx):
                """beta matmuls reuse the gate banks (after sigmoid), tt."""
                p_t, sT16 = ctx["p_t"], ctx["sT16"]
                for c0, c1 in ((0, 512), (512, 768)):
                    for k in range(3):
                        nc.tensor.matmul(
                            p_t[:, c0:c1], sT16[:, k], wb_t[:, k, c0:c1],
                            start=(k == 0), stop=False,
                        )
                ttv = work.tile([P, CA], f16, tag="tt", bufs=6)
                nc.vector.tensor_tensor(
                    out=ttv, in0=ctx["an"], in1=ctx["gate"], op=OP.mult
                )
                ctx["ttv"] = ttv

            def stage_b(ctx, m):
                """PE-add tt into beta psum, then ACT evict psum -> fp16."""
                p_t, ttv, j = ctx["p_t"], ctx["ttv"], ctx["j"]
                for c0, c1 in ((0, 512), (512, 768)):
                    nc.tensor.matmul(
                        p_t[:, c0:c1], ident, ttv[:, c0:c1],
                        start=False, stop=True,
                    )
                o_t = oio.tile([P, CA], f16, tag="o_t", bufs=6)
                nc.scalar.activation(
                    out=o_t, in_=p_t[:, 0:768], func=AF.Copy
                )
                nc.sync.dma_start(out=o_v[m, :, j], in_=o_t)

            do_stats(st_cur)

            # Software pipeline, ~3 subtiles in flight:
            #   a1(j) gate side | a3(j-1) beta (banks freed by sigma(j-1))
            #   | a2(j) sigmoid | b(j-2) add + evict
            p1 = None   # (st, ctx) after a1/a2, awaiting a3
            p2 = None   # (ctx, m) after a3, awaiting b
            for m in range(NMACRO):
                st = st_cur
                st_next2 = load(m + 2) if m + 2 < NMACRO else None  # 3-deep via bufs
                for j in range(J):
                    ctx = stage_a1(st, j)
                    if p1 is not None:
                        stage_a3(p1[1])
                    stage_a2(st, ctx)
                    if p2 is not None:
                        stage_b(*p2)
                    p2 = (p1[1], p1[2]) if p1 is not None else None
                    p1 = (st, ctx, m)
                    if st_next is not None:
                        stats_j(st_next, j)
                        if j == J - 1:
                            stats_fin(st_next)
                st_cur, st_next = st_next, st_next2
            stage_a3(p1[1])
            if p2 is not None:
                stage_b(*p2)
            stage_b(p1[1], p1[2])

    nc.finalize()
    return nc


def _get_nc():
    if "nc" not in _CACHE:
        _CACHE["nc"] = _build()
    return _CACHE["nc"]


def _pack_weights(ln_s_weight, w_gamma, b_gamma, w_beta):
    f8 = ml_dtypes.float8_e4m3
    lnw = np.asarray(ln_s_weight, np.float32)
    wgT = np.ascontiguousarray(
        (np.asarray(w_gamma, np.float32) * lnw[None, :]).T
    )  # [384, 768]
    wbT = np.ascontiguousarray(
        (np.asarray(w_beta, np.float32) * lnw[None, :]).T
    )
    wg8 = np.zeros((P, 2, 2, CA), np.float32)
    for pair in range(2):
        for slot in range(2):
            c0 = pair * 256 + slot * 128
            if c0 < CS:
                wg8[:, pair, slot, :] = wgT[c0 : c0 + 128, :]
    wg8[0, 1, 1, :] = np.asarray(b_gamma, np.float32)
    wg8 = wg8.astype(f8)
    wb16 = np.ascontiguousarray(
        wbT.reshape(3, P, CA).transpose(1, 0, 2)
    ).astype(np.float16)
    return wg8, wb16


def _prep_inputs(a, s, ln_s_weight, w_gamma, b_gamma, w_beta):
    a2 = np.asarray(a, np.float32).reshape(B * N, CA).astype(np.float16)
    s2 = np.asarray(s, np.float32).reshape(B * N, CS).astype(np.float16)
    wg8, wb16 = _pack_weights(ln_s_weight, w_gamma, b_gamma, w_beta)
    in_maps = []
    for i in range(NCORES):
        in_maps.append(
            {
                "a": a2[i * T : (i + 1) * T],
                "s": s2[i * T : (i + 1) * T],
                "wg8": wg8,
                "wb16": wb16,
            }
        )
    return in_maps


def run(a, s, ln_s_weight, w_gamma, b_gamma, w_beta, trace=False, tmpdir=None):
    """Run on 8 NeuronCores; returns (output, BassKernelResults)."""
    from concourse import bass_utils

    nc = _get_nc()
    in_maps = _prep_inputs(a, s, ln_s_weight, w_gamma, b_gamma, w_beta)
    res = bass_utils.run_bass_kernel_spmd(
        nc, in_maps, core_ids=list(range(NCORES)), trace=trace, tmpdir=tmpdir
    )
    out = np.concatenate([np.asarray(r["out"]) for r in res.results], axis=0)
    return out.reshape(B, N, CA).astype(np.float32), res


def kernel(a, s, ln_s_weight, w_gamma, b_gamma, w_beta):
    out, _ = run(a, s, ln_s_weight, w_gamma, b_gamma, w_beta, trace=False)
    return out
